# revision 1
# baseline (speedup 1.0000x reference)
"""AGATCellWithMLP Trainium2 kernel: 8-core data-parallel over batch B.

Each core handles one graph. Host-side we permute each graph's nodes so the
512 "selected" nodes (nodes_n order) come first; the kernel then only computes
attention/MLP query rows for those 512 nodes (keys/values span all 1024).
All heavy tensors are kept in transposed [channel, node] layout so the
attention probabilities never need an on-chip transpose; the softmax
denominator is produced by an extra ones-column in the V matmul.

The combined-feature dim C=129 is split as 128 PE-contraction rows plus a
rank-1 "channel 128" correction (fused DVE op or tiny extra K-chunk), so
every big matmul needs only ONE K-chunk instead of two.
"""

import sys

sys.path.insert(0, "/opt/trn_rl_repo")

from contextlib import ExitStack

import numpy as np
import ml_dtypes

import concourse.bass as bass
import concourse.bacc as bacc
import concourse.tile as tile
from concourse import mybir
from concourse.bass_utils import run_bass_kernel_spmd
from concourse.masks import make_identity
from concourse.bass import ts

P = 128
B, N, D, H, QV = 8, 1024, 64, 4, 32
C = 2 * D + 1           # 129
KD = C // 8             # 16
NQ = 512                # selected nodes (queries) per graph
CX, CH = D + 1, D       # 65 + 64 channel split (x | h)
NEG = -9e15
F32 = mybir.dt.float32
BF16 = mybir.dt.bfloat16
AX = mybir.AxisListType
ALU = mybir.AluOpType
ACTF = mybir.ActivationFunctionType

NT = N // P             # 8 key tiles
MT = NQ // P            # 4 query tiles


def build_graph():
    nc = bacc.Bacc()

    xp = nc.declare_dram_parameter("xp", [N, CX], F32, False)
    hp = nc.declare_dram_parameter("hp", [N, CH], F32, False)
    adjT = nc.declare_dram_parameter("adjT", [N, NQ], F32, False)
    qv = nc.declare_dram_parameter("qv", [NQ, QV], F32, False)
    # per-head q|k weights packed [C, 48]: k at cols 0:16, q (pre-scaled) at 32:48
    wqk = nc.declare_dram_parameter("wqk", [H, C, 48], BF16, False)
    bqk = nc.declare_dram_parameter("bqk", [H, 48], F32, False)
    wv = nc.declare_dram_parameter("wv", [H, C, C], BF16, False)
    w1 = nc.declare_dram_parameter("w1", [H, C, C], BF16, False)
    b1 = nc.declare_dram_parameter("b1", [C], F32, False)
    w2 = nc.declare_dram_parameter("w2", [C, C], BF16, False)
    b2 = nc.declare_dram_parameter("b2", [C], F32, False)
    wall = nc.declare_dram_parameter("wall", [3, P, QV * D], BF16, False)
    w128 = nc.declare_dram_parameter("w128", [3, QV, D], BF16, False)
    ball = nc.declare_dram_parameter("ball", [3, QV, D], BF16, False)
    out = nc.declare_dram_parameter("out", [NQ, D], F32, True)

    with tile.TileContext(nc) as tc, ExitStack() as ctx:
        sing = ctx.enter_context(tc.tile_pool(name="sing", bufs=1))
        work = ctx.enter_context(tc.tile_pool(name="work", bufs=3))
        pexp_pool = ctx.enter_context(tc.tile_pool(name="pexp", bufs=4))
        vpool = ctx.enter_context(tc.tile_pool(name="vpool", bufs=4))
        psA = ctx.enter_context(tc.tile_pool(name="psA", bufs=3, space="PSUM"))
        psB = ctx.enter_context(tc.tile_pool(name="psB", bufs=2, space="PSUM"))
        psC = ctx.enter_context(tc.tile_pool(name="psC", bufs=1, space="PSUM"))
        psS = ctx.enter_context(tc.tile_pool(name="psS", bufs=2, space="PSUM"))

        def pA(p_, f_):
            return psA.tile([p_, f_], F32, tag="pA", name="pA")

        def pAb(p_, f_):
            return psA.tile([p_, f_], BF16, tag="pA", name="pA")

        def pB(p_, f_):
            return psB.tile([p_, f_], F32, tag="pB", name="pB")

        def pC(p_, f_):
            return psC.tile([p_, f_], F32, tag="pC", name="pC")

        def pS(p_, f_):
            return psS.tile([p_, f_], F32, tag="pS", name="pS")

        ident = sing.tile([P, P], BF16)
        make_identity(nc, ident[:])
        identf = sing.tile([P, P], F32)
        make_identity(nc, identf[:])
        ones_row = sing.tile([1, P], BF16)
        nc.vector.memset(ones_row[:], 1.0)

        # ---- inputs on the gpsimd queue (parallel with weight DMAs) ----
        comb = [sing.tile([P, C], F32, tag=f"comb{i}", name=f"comb{i}")
                for i in range(NT)]
        for i in range(NT):
            nc.gpsimd.dma_start(comb[i][:, 0:CX], xp[ts(i, P), :])
            nc.gpsimd.dma_start(comb[i][:, CX:C], hp[ts(i, P), :])
        qv_sb = [sing.tile([P, QV], F32, tag=f"qv{j}", name=f"qv{j}")
                 for j in range(MT)]
        for j in range(MT):
            nc.gpsimd.dma_start(qv_sb[j][:], qv[ts(j, P), :])
        adjT_sb = [sing.tile([P, NQ], F32, tag=f"adjT{i}", name=f"adjT{i}")
                   for i in range(NT)]
        for i in range(NT):
            nc.gpsimd.dma_start(adjT_sb[i][:], adjT[ts(i, P), :])

        # ---- weights on the sync queue ----
        wqk_sb = [sing.tile([P, 48], BF16, tag=f"wqk{h}", name=f"wqk{h}")
                  for h in range(H)]
        wqkL = [sing.tile([1, 48], BF16, tag=f"wqkL{h}", name=f"wqkL{h}")
                for h in range(H)]
        wv_sb = [sing.tile([P, C], BF16, tag=f"wv{h}", name=f"wv{h}")
                 for h in range(H)]
        wvL = [sing.tile([1, C], BF16, tag=f"wvL{h}", name=f"wvL{h}")
               for h in range(H)]
        w1_sb = [sing.tile([P, C], BF16, tag=f"w1{h}", name=f"w1{h}")
                 for h in range(H)]
        w1L = sing.tile([H, C], BF16, name="w1L")
        for h in range(H):
            nc.sync.dma_start(wqk_sb[h][:], wqk[h, 0:P, :])
            nc.sync.dma_start(wqkL[h][:], wqk[h, P:C, :])
            nc.sync.dma_start(wv_sb[h][:], wv[h, 0:P, :])
            nc.sync.dma_start(wvL[h][:], wv[h, P:C, :])
            nc.sync.dma_start(w1_sb[h][:], w1[h, 0:P, :])
            nc.sync.dma_start(w1L[h:h + 1, :], w1[h, P:C, :])
        w2_sb = (sing.tile([CX, C], BF16, tag="w2x", name="w2x"),
                 sing.tile([CH, C], BF16, tag="w2h", name="w2h"))
        nc.sync.dma_start(w2_sb[0][:], w2[0:CX, :])
        nc.sync.dma_start(w2_sb[1][:], w2[CX:C, :])
        bqk_sb = [sing.tile([48, 1], F32, tag=f"bqk{h}", name=f"bqk{h}")
                  for h in range(H)]
        for h in range(H):
            nc.sync.dma_start(bqk_sb[h][:], bqk[h, :, None])
        b1_sb = (sing.tile([CX, 1], F32, tag="b1x", name="b1x"),
                 sing.tile([CH, 1], F32, tag="b1h", name="b1h"))
        nc.sync.dma_start(b1_sb[0][:], b1[0:CX, None])
        nc.sync.dma_start(b1_sb[1][:], b1[CX:C, None])
        b2_sb = (sing.tile([CX, 1], F32, tag="b2x", name="b2x"),
                 sing.tile([CH, 1], F32, tag="b2h", name="b2h"))
        nc.sync.dma_start(b2_sb[0][:], b2[0:CX, None])
        nc.sync.dma_start(b2_sb[1][:], b2[CX:C, None])
        wall_sb = [sing.tile([P, QV * D], BF16, tag=f"wall{i}", name=f"wall{i}")
                   for i in range(3)]
        w128_sb = [sing.tile([QV, D], BF16, tag=f"w128_{i}", name=f"w128_{i}")
                   for i in range(3)]
        ball_sb = [sing.tile([QV, D], BF16, tag=f"ball{i}", name=f"ball{i}")
                   for i in range(3)]
        for i in range(3):
            nc.sync.dma_start(wall_sb[i][:], wall[i, :, :])
            nc.sync.dma_start(w128_sb[i][:], w128[i, :, :])
            nc.sync.dma_start(ball_sb[i][:], ball[i, :, :])

        # ---- transposed combined: combT [128, 1024] (c 0:128) + c128T [1, 1024]
        combT = sing.tile([P, N], BF16)
        c128T = sing.tile([1, N], BF16)
        for i in range(NT):
            cb = work.tile([P, C], BF16, tag="cb", name="cb")
            nc.scalar.copy(cb[:], comb[i][:])
            pt = pAb(P, P)
            nc.tensor.transpose(pt[:], cb[:, 0:P], ident[:])
            nc.scalar.copy(combT[:, ts(i, P)], pt[:])
            pl = pAb(1, P)
            nc.tensor.transpose(pl[:], cb[:, P:C], ident[:])
            nc.scalar.copy(c128T[:, ts(i, P)], pl[:])
        # qv transposed [32, 512]
        qvT = sing.tile([QV, NQ], BF16)
        for j in range(MT):
            qb16 = work.tile([P, QV], BF16, tag="qb16", name="qb16")
            nc.scalar.copy(qb16[:], qv_sb[j][:])
            pq = pAb(QV, P)
            nc.tensor.transpose(pq[:], qb16[:], ident[:])
            nc.scalar.copy(qvT[:, ts(j, P)], pq[:])

        # residual rows for the cf h-group: channels 65:129 = combT[65:128]+c128T
        combresH = sing.tile([CH, NQ], BF16)
        nc.gpsimd.dma_start(combresH[0:CH - 1, :], combT[CX:P, 0:NQ])
        nc.gpsimd.dma_start(combresH[CH - 1:CH, :], c128T[:, 0:NQ])

        # ---- attention heads ----
        acT = []
        for h in range(H):
            # q,k in one matmul pair: psum rows 0:16 = k, rows 32:48 = q
            kT = work.tile([KD, N], BF16, tag="kT", name="kT")
            qT = work.tile([KD, NQ], BF16, tag="qT", name="qT")
            for half in range(2):
                pk = pC(48, NQ)
                nc.tensor.matmul(pk[:], wqk_sb[h][:], combT[:, ts(half, NQ)],
                                 start=True, stop=False)
                nc.tensor.matmul(pk[:], wqkL[h][:], c128T[:, ts(half, NQ)],
                                 start=False, stop=True)
                nc.scalar.activation(kT[:, ts(half, NQ)], pk[0:KD, :],
                                     ACTF.Identity, bias=bqk_sb[h][0:KD, :])
                if half == 0:
                    nc.scalar.activation(qT[:], pk[32:48, :], ACTF.Identity,
                                         bias=bqk_sb[h][32:48, :])

            # broadcast Wv's channel-128 row across partitions (for the V fixup)
            pvl = pA(P, C)
            nc.tensor.matmul(pvl[:], ones_row[:], wvL[h][:], start=True, stop=True)
            wvLrep = work.tile([P, C], BF16, tag="wvLrep", name="wvLrep")
            nc.vector.tensor_copy(wvLrep[:], pvl[:])

            phg1 = pB(CX, NQ)
            phg2 = pB(CX, NQ)
            for i in range(NT):
                # V tile [128, 130]: cols 0:129 = combined @ Wv, col 129 = 1.0
                vt = vpool.tile([P, C + 1], BF16, tag="vt", name="vt")
                pv = pA(P, C)
                nc.tensor.matmul(pv[:], combT[:, ts(i, P)], wv_sb[h][:],
                                 start=True, stop=True)
                # channel-128 rank-1 fixup fused with the PSUM->SBUF copy
                nc.vector.scalar_tensor_tensor(
                    vt[:, 0:C], wvLrep[:], comb[i][:, P:C], pv[:],
                    op0=ALU.mult, op1=ALU.add)
                nc.gpsimd.memset(vt[:, C:C + 1], 1.0)

                # scores^T tile: [128 keys, 512 queries]
                ps = pS(P, NQ)
                nc.tensor.matmul(ps[:], kT[:, ts(i, P)], qT[:],
                                 start=True, stop=True)
                # mask first (leaky(s+m) == leaky(s)+m for m in {0, -9e15}),
                # then leaky = max(t, 0.2t) in one fused DVE op, then exp.
                t0 = work.tile([P, NQ], F32, tag="t0", name="t0")
                nc.vector.tensor_tensor(t0[:], ps[:], adjT_sb[i][:], ALU.add)
                sm = work.tile([P, NQ], F32, tag="sm", name="sm")
                nc.vector.scalar_tensor_tensor(sm[:], t0[:], 0.2, t0[:],
                                               op0=ALU.mult, op1=ALU.max)
                pe = pexp_pool.tile([P, NQ], BF16, tag="pe", name="pe")
                nc.scalar.activation(pe[:], sm[:], ACTF.Exp)

                nc.tensor.matmul(phg1[:], vt[:, 0:CX], pe[:],
                                 start=(i == 0), stop=(i == NT - 1))
                nc.tensor.matmul(phg2[:], vt[:, CX:C + 1], pe[:],
                                 start=(i == 0), stop=(i == NT - 1))

            # stash raw hp (and the denominator row); normalization is batched
            ar1 = sing.tile([CX, NQ], F32, tag=f"araw{h}x", name=f"araw{h}x")
            ar2 = sing.tile([CX, NQ], F32, tag=f"araw{h}h", name=f"araw{h}h")
            nc.scalar.copy(ar1[:], phg1[:])
            nc.scalar.copy(ar2[:], phg2[:])
            acT.append((ar1, ar2))

        # ---- batched softmax normalization: ONE reciprocal for all 4 heads.
        # Engine reads/writes need base partition 0/32/64, so row scatter and
        # gather go through tiny SBUF-SBUF DMAs.
        rcat = sing.tile([H, NQ], F32, name="rcat")
        for h in range(H):
            nc.gpsimd.dma_start(rcat[h:h + 1, :], acT[h][1][CX - 1:CX, :])
        rinv4 = sing.tile([H, NQ], F32, name="rinv4")
        nc.vector.reciprocal(rinv4[:], rcat[:])
        rtmp = [sing.tile([1, NQ], F32, tag=f"rtmp{h}", name=f"rtmp{h}")
                for h in range(H)]
        for h in range(H):
            nc.gpsimd.dma_start(rtmp[h][:], rinv4[h:h + 1, :])
        # normalized attn_cat^T per head: a128 [128, 512] (c 0:128) in one tile
        # (rows 65:128 arrive via DMA), last channels gathered into aL [4, 512]
        a128 = []
        aL = sing.tile([H, NQ], BF16, name="aL")
        for h in range(H):
            ar1, ar2 = acT[h]
            rinv_bf = work.tile([1, NQ], BF16, tag="rinv_bf", name="rinv_bf")
            nc.scalar.copy(rinv_bf[:], rtmp[h][:])
            pbc = pA(P, NQ)
            nc.tensor.matmul(pbc[:], ones_row[:], rinv_bf[:], start=True, stop=True)
            rb = work.tile([P, NQ], F32, tag="rb", name="rb")
            nc.scalar.copy(rb[:], pbc[:])
            ah = sing.tile([P, NQ], BF16, tag=f"a128_{h}", name=f"a128_{h}")
            nc.vector.tensor_tensor(ah[0:CX, :], ar1[:], rb[0:CX, :], ALU.mult)
            a2t = sing.tile([CH, NQ], BF16, tag=f"a2t{h}", name=f"a2t{h}")
            nc.vector.tensor_tensor(a2t[:], ar2[0:CH, :], rb[0:CH, :], ALU.mult)
            nc.gpsimd.dma_start(ah[CX:P, :], a2t[0:CH - 1, :])
            nc.gpsimd.dma_start(aL[h:h + 1, :], a2t[CH - 1:CH, :])
            a128.append(ah)

        # ---- MLP (transposed): m1 = relu(W1^T ac + b1); cf = W2^T m1 + b2 + comb
        m1T = (work.tile([CX, NQ], BF16, tag="m1x", name="m1x"),
               work.tile([CH, NQ], BF16, tag="m1h", name="m1h"))
        for g, (off, ln) in enumerate(((0, CX), (CX, CH))):
            pm = pB(CX, NQ)
            for h in range(H):
                nc.tensor.matmul(pm[0:ln, :], w1_sb[h][:, off:off + ln],
                                 a128[h][:], start=(h == 0), stop=False)
            nc.tensor.matmul(pm[0:ln, :], w1L[:, off:off + ln], aL[:],
                             start=False, stop=True)
            nc.scalar.activation(m1T[g][:], pm[0:ln, :], ACTF.Relu, bias=b1_sb[g][:])
        # cf128 [128, 512] = combined_final^T channels 0:128; cl_row = channel 128
        cf128 = sing.tile([P, NQ], BF16, name="cf128")
        cfh = work.tile([CH, NQ], BF16, tag="cfh", name="cfh")
        for g, (off, ln) in enumerate(((0, CX), (CX, CH))):
            pm = pB(CX, NQ)
            nc.tensor.matmul(pm[0:ln, :], w2_sb[0][:, off:off + ln], m1T[0][:],
                             start=True, stop=False)
            nc.tensor.matmul(pm[0:ln, :], w2_sb[1][:, off:off + ln], m1T[1][:],
                             start=False, stop=True)
            dst = cf128[0:CX, :] if g == 0 else cfh[:]
            src = combT[0:CX, 0:NQ] if g == 0 else combresH[:]
            nc.vector.scalar_tensor_tensor(dst, pm[0:ln, :], b2_sb[g][:], src,
                                           op0=ALU.add, op1=ALU.add)
        nc.gpsimd.dma_start(cf128[CX:P, :], cfh[0:CH - 1, :])
        cl_row = sing.tile([1, NQ], BF16, name="cl_row")
        nc.gpsimd.dma_start(cl_row[:], cfh[CH - 1:CH, :])
        # natural-layout channel-128 scalars [128, 1] per query tile
        scl = [sing.tile([P, 1], F32, tag=f"scl{j}", name=f"scl{j}")
               for j in range(MT)]
        for j in range(MT):
            pt = pAb(P, 1)
            nc.tensor.transpose(pt[:], cl_row[:, ts(j, P)], ident[0:1, 0:1])
            nc.scalar.copy(scl[j][:], pt[:])

        # ---- hypernetwork stage ----
        def hyper(idx, sel128, selL, func, outs):
            """outs[j][128,64] = func(sel @ W_all + selL*(qv@W128) + qv @ b).

            wall[c, o*QV+d] = W[d, c, o] for c<128; the c=128 row is handled
            via the per-partition scalar selL and qv @ W128.
            """
            og = NQ // QV  # o-values per 512-wide chunk (16)
            for j in range(MT):
                pbn = pC(P, 2 * D)
                nc.tensor.matmul(pbn[:, 0:D], qvT[:, ts(j, P)], ball_sb[idx][:],
                                 start=True, stop=True)
                nc.tensor.matmul(pbn[:, D:2 * D], qvT[:, ts(j, P)],
                                 w128_sb[idx][:], start=True, stop=True)
                o1 = work.tile([P, D], F32, tag="o1", name="o1")
                for nch in range(4):  # 512-wide chunks of the 2048 (o,d) axis
                    pt = pA(P, NQ)
                    nc.tensor.matmul(pt[:], sel128[:, ts(j, P)],
                                     wall_sb[idx][:, ts(nch, NQ)],
                                     start=True, stop=True)
                    prod = work.tile([P, NQ], BF16, tag="prod", name="prod")
                    qb = qv_sb[j][:, None, :].to_broadcast((P, og, QV))
                    nc.vector.tensor_tensor(
                        prod[:].rearrange("p (a b) -> p a b", b=QV),
                        pt[:].rearrange("p (a b) -> p a b", b=QV),
                        qb, ALU.mult)
                    nc.vector.tensor_reduce(
                        o1[:, ts(nch, og)],
                        prod[:].rearrange("p (a b) -> p a b", b=QV),
                        axis=AX.X, op=ALU.add)
                nc.vector.tensor_tensor(o1[:], o1[:], pbn[:, 0:D], ALU.add)
                nc.vector.scalar_tensor_tensor(o1[:], pbn[:, D:2 * D], selL[j],
                                               o1[:], op0=ALU.mult, op1=ALU.add)
                nc.scalar.activation(outs[j][:], o1[:], func)

        r_t = [work.tile([P, D], F32, tag=f"r{j}", name=f"r{j}") for j in range(MT)]
        u_t = [work.tile([P, D], F32, tag=f"u{j}", name=f"u{j}") for j in range(MT)]
        c_t = [work.tile([P, D], F32, tag=f"c{j}", name=f"c{j}") for j in range(MT)]
        hyper(0, cf128, [scl[j][:] for j in range(MT)], ACTF.Sigmoid, r_t)
        hyper(1, cf128, [scl[j][:] for j in range(MT)], ACTF.Sigmoid, u_t)

        # h_new = r * h_sel; selc128 = [x (65) | hn (63)]; last hn channel is
        # the per-partition scalar
        selc128 = sing.tile([P, NQ], BF16, name="selc128")
        nc.scalar.copy(selc128[0:CX, :], combT[0:CX, 0:NQ])
        hnT = sing.tile([CH, NQ], BF16)
        hn_t = []
        for j in range(MT):
            hn = work.tile([P, D], F32, tag=f"hn{j}", name=f"hn{j}")
            nc.vector.tensor_tensor(hn[:], r_t[j][:], comb[j][:, CX:C], ALU.mult)
            hn_t.append(hn)
            pt = pA(CH, P)
            nc.tensor.transpose(pt[:], hn[:], identf[:])
            nc.scalar.copy(hnT[:, ts(j, P)], pt[:])
        nc.gpsimd.dma_start(selc128[CX:P, :], hnT[0:CH - 1, :])
        hyper(2, selc128, [hn_t[j][:, D - 1:D] for j in range(MT)],
              ACTF.Tanh, c_t)

        # out = h_new + u * (cand - h_new)
        for j in range(MT):
            t1 = work.tile([P, D], F32, tag="t1", name="t1")
            nc.vector.tensor_tensor(t1[:], c_t[j][:], hn_t[j][:], ALU.subtract)
            nc.vector.tensor_tensor(t1[:], t1[:], u_t[j][:], ALU.mult)
            nc.vector.tensor_tensor(t1[:], t1[:], hn_t[j][:], ALU.add)
            nc.sync.dma_start(out[ts(j, P), :], t1[:])

    return nc


_NC_CACHE = None


def _get_nc():
    global _NC_CACHE
    if _NC_CACHE is None:
        _NC_CACHE = build_graph()
        if not _NC_CACHE.is_finalized():
            _NC_CACHE.finalize()
    return _NC_CACHE


def _prep_core(b, x, h, query_vectors, adj, nodes_n, shared):
    idx = nodes_n[b * NQ:(b + 1) * NQ].astype(np.int64)
    rest = np.setdiff1d(np.arange(N, dtype=np.int64), idx)
    perm = np.concatenate([idx, rest])
    d = dict(shared)
    d["xp"] = np.ascontiguousarray(x[b][perm])
    d["hp"] = np.ascontiguousarray(h[b][perm])
    d["adjT"] = np.ascontiguousarray(
        np.where(adj[np.ix_(idx, perm)] != 0, np.float32(0), np.float32(NEG)).T)
    d["qv"] = np.ascontiguousarray(query_vectors[b * NQ:(b + 1) * NQ])
    return d


def _prep_shared(Wq, bq, Wk, bk, Wv, bv, W1, b1, W2, b2, Wr, br, Wu, bu, Wc, bc):
    bf = ml_dtypes.bfloat16
    W1r = np.asarray(W1, np.float32).reshape(H, C, C)
    b1_eff = np.asarray(b1, np.float32) + sum(
        np.asarray(bv, np.float32)[hh] @ W1r[hh] for hh in range(H))
    # per-head [C, 48]: k at 0:16, q/4 at 32:48; bias likewise
    wqk_np = np.zeros((H, C, 48), np.float32)
    wqk_np[:, :, 0:16] = np.asarray(Wk, np.float32)
    wqk_np[:, :, 32:48] = np.asarray(Wq, np.float32) * 0.25
    bqk_np = np.zeros((H, 48), np.float32)
    bqk_np[:, 0:16] = np.asarray(bk, np.float32)
    bqk_np[:, 32:48] = np.asarray(bq, np.float32) * 0.25
    packW = lambda W: np.ascontiguousarray(
        np.transpose(np.asarray(W, np.float32), (1, 2, 0)).reshape(C, D * QV)[0:P])
    lastW = lambda W: np.ascontiguousarray(np.asarray(W, np.float32)[:, P, :])
    return dict(
        wqk=np.ascontiguousarray(wqk_np.astype(bf)),
        bqk=np.ascontiguousarray(bqk_np),
        wv=np.ascontiguousarray(np.asarray(Wv, np.float32).astype(bf)),
        w1=np.ascontiguousarray(W1r.astype(bf)),
        b1=np.ascontiguousarray(b1_eff),
        w2=np.ascontiguousarray(np.asarray(W2, np.float32).astype(bf)),
        b2=np.ascontiguousarray(np.asarray(b2, np.float32)),
        wall=np.ascontiguousarray(np.stack(
            [packW(Wr), packW(Wu), packW(Wc)]).astype(bf)),
        w128=np.ascontiguousarray(np.stack(
            [lastW(Wr), lastW(Wu), lastW(Wc)]).astype(bf)),
        ball=np.ascontiguousarray(np.stack([
            np.asarray(br, np.float32), np.asarray(bu, np.float32),
            np.asarray(bc, np.float32)]).astype(bf)),
    )


def make_in_maps(x, h, query_vectors, adj, nodes_b, nodes_n, **weights):
    x = np.asarray(x, np.float32)
    h = np.asarray(h, np.float32)
    query_vectors = np.asarray(query_vectors, np.float32)
    adj = np.asarray(adj)
    nodes_n = np.asarray(nodes_n)
    shared = _prep_shared(**weights)
    return [_prep_core(b, x, h, query_vectors, adj, nodes_n, shared)
            for b in range(B)]


def kernel(x, h, query_vectors, adj, nodes_b, nodes_n,
           Wq, bq, Wk, bk, Wv, bv, W1, b1, W2, b2,
           Wr, br, Wu, bu, Wc, bc):
    in_maps = make_in_maps(
        x, h, query_vectors, adj, nodes_b, nodes_n,
        Wq=Wq, bq=bq, Wk=Wk, bk=bk, Wv=Wv, bv=bv, W1=W1, b1=b1, W2=W2, b2=b2,
        Wr=Wr, br=br, Wu=Wu, bu=bu, Wc=Wc, bc=bc)
    nc = _get_nc()
    res = run_bass_kernel_spmd(nc, in_maps, list(range(B)))
    outs = [np.asarray(res.results[b]["out"], np.float32) for b in range(B)]
    return np.concatenate(outs, axis=0)



# revision 25
# speedup vs baseline: 1.4632x; 1.4632x over previous
"""AGATCellWithMLP Trainium2 kernel: 8-core data-parallel over batch B.

v2 design (one graph per core, everything transposed [channel, node]):
 - Host sends combT pre-transposed bf16 with channels reordered [h | x] so
   that every on-chip partition slice lands on a 0/32/64 base.
 - W1 is folded into Wv host-side (U_h = Wv_h @ W1_h), so the attention
   numerator matmuls directly produce the MLP hidden pre-activations; the
   bv@W1 term is exact via the softmax denominator (1^T P D^-1 = 1).
 - qk for a head-pair runs as one packed matmul; biases and the 1/sqrt(K)
   scale ride an appended ones-row of combT.
 - leaky_relu on ACT (Prelu, alpha=0.2) / DVE (tunable split); exp on ACT;
   the adjacency mask is multiplicative 0/1 bf16 after exp (2x DVE mode).
 - Numerator + denominator + channel-128 accumulate in PSUM across the 8
   key tiles per head; normalization uses reciprocal_approx_fast + gpsimd
   partition_broadcast, folded per head-pair to keep PSUM inside 8 banks.
 - Hypernetwork uses the bilinear z-trick: zT[(i,d), m] = selT[i,m]*qvT[d,m]
   built by bf16 2x DVE TTs against host-prebroadcast qb rows; the whole
   per-query einsum is then 34 PSUM-accumulated matmuls per gate pair
   (bias and c128-channel terms are two more accumulating matmuls).
"""

import sys

sys.path.insert(0, "/opt/trn_rl_repo")

from contextlib import ExitStack

import numpy as np
import ml_dtypes

import concourse.bass as bass
import concourse.bacc as bacc
import concourse.tile as tile
from concourse import mybir
from concourse import bass_isa
from concourse.bass_utils import run_bass_kernel_spmd
from concourse.masks import make_identity
from concourse.bass import ts

P = 128
B, N, D, H, QV = 8, 1024, 64, 4, 32
C = 2 * D + 1            # 129
KD = C // 8              # 16
NQ = 512                 # selected nodes (queries) per graph
F32 = mybir.dt.float32
BF16 = mybir.dt.bfloat16
AX = mybir.AxisListType
ALU = mybir.AluOpType
ACTF = mybir.ActivationFunctionType

NT = N // P              # 8 key tiles

# leaky on ACT (Prelu) for the first ACT_LEAKY of 16 (pair, tile) units;
# the rest use a DVE STT. Sim cannot run Prelu -> test.py flips
# USE_LRELU[0]=False to force the DVE path everywhere.
USE_LRELU = [True]
ACT_LEAKY = 12


def build_graph(hw_leaky=True):
    nc = bacc.Bacc()

    combT_d = nc.declare_dram_parameter("combT", [P, N], BF16, False)
    c128_d = nc.declare_dram_parameter("c128", [1, N], BF16, False)
    kqw_d = nc.declare_dram_parameter("kqw", [P, 256], BF16, False)
    kqwL_d = nc.declare_dram_parameter("kqwL", [1, 256], BF16, False)
    bkq_d = nc.declare_dram_parameter("bkq", [KD, 2 * H], F32, False)
    wv1a_d = nc.declare_dram_parameter("wv1a", [P, 516], BF16, False)
    wv1b_d = nc.declare_dram_parameter("wv1b", [1, 516], BF16, False)
    adjT_d = nc.declare_dram_parameter("adjT", [P, NT * NQ], BF16, False)
    w2a_d = nc.declare_dram_parameter("w2a", [P, C], BF16, False)
    w2b_d = nc.declare_dram_parameter("w2b", [1, C], BF16, False)
    bias_d = nc.declare_dram_parameter("biases", [P, 4], F32, False)
    biasL_d = nc.declare_dram_parameter("biasesL", [1, 4], F32, False)
    qvT_d = nc.declare_dram_parameter("qvT", [QV, NQ], BF16, False)
    qb_d = nc.declare_dram_parameter("qb", [P, QV * NQ], BF16, False)
    wzru_d = nc.declare_dram_parameter("wzru", [P, QV * P], BF16, False)
    wzc_d = nc.declare_dram_parameter("wzc", [P, QV * D], BF16, False)
    # [32, 384]: cols 0:128 wzruL, 128:256 bru, 256:320 wzcL, 320:384 bc
    small_d = nc.declare_dram_parameter("smalls", [QV, 384], BF16, False)
    out_d = nc.declare_dram_parameter("out", [NQ, D], F32, True)

    with tile.TileContext(nc) as tc, ExitStack() as ctx:
        sing = ctx.enter_context(tc.tile_pool(name="sing", bufs=1))
        smp = ctx.enter_context(tc.tile_pool(name="smp", bufs=2))
        pep = ctx.enter_context(tc.tile_pool(name="pep", bufs=3))
        work = ctx.enter_context(tc.tile_pool(name="work", bufs=3))
        # PSUM budget (8 banks): psS 2x[128,1024] = 4, psY 2x[128,512] = 2,
        # psE 2x[2,512] = 2.  qk/V/MLP/hyper psums share these pools.
        psS = ctx.enter_context(tc.tile_pool(name="psS", bufs=2, space="PSUM"))
        psY = ctx.enter_context(tc.tile_pool(name="psY", bufs=2, space="PSUM"))
        psE = ctx.enter_context(tc.tile_pool(name="psE", bufs=2, space="PSUM"))

        identf = sing.tile([P, P], F32)
        make_identity(nc, identf[:])
        zeroN = sing.tile([KD, N], F32, name="zeroN")
        nc.gpsimd.memset(zeroN[:], 0.0)

        # ---------------- input DMAs (sync queue, rough use order) --------
        combT = sing.tile([P, N], BF16)
        nc.sync.dma_start(combT[:], combT_d[:, :])
        cxr = sing.tile([1, N], BF16)            # channel-128 row (last x)
        nc.sync.dma_start(cxr[:], c128_d[:, :])
        kqw = sing.tile([P, 256], BF16)
        kqwL = sing.tile([1, 256], BF16)
        bkq = sing.tile([KD, 2 * H], F32)
        nc.sync.dma_start(kqw[:], kqw_d[:, :])
        nc.sync.dma_start(kqwL[:], kqwL_d[:, :])
        nc.sync.dma_start(bkq[:], bkq_d[:, :])
        wv1a = sing.tile([P, 516], BF16)
        wv1b = sing.tile([1, 516], BF16)
        nc.sync.dma_start(wv1a[:], wv1a_d[:, :])
        nc.sync.dma_start(wv1b[:], wv1b_d[:, :])
        adjT = sing.tile([P, NT * NQ], BF16)
        nc.sync.dma_start(adjT[:], adjT_d[:, :])
        w2a = sing.tile([P, C], BF16)
        w2b = sing.tile([1, C], BF16)
        nc.sync.dma_start(w2a[:], w2a_d[:, :])
        nc.sync.dma_start(w2b[:], w2b_d[:, :])
        biases = sing.tile([P, 4], F32)   # cols: 0 = b1, 1 = b2
        biasesL = sing.tile([1, 4], F32)
        nc.sync.dma_start(biases[:], bias_d[:, :])
        nc.sync.dma_start(biasesL[:], biasL_d[:, :])
        qvT = sing.tile([QV, NQ], BF16)
        nc.sync.dma_start(qvT[:], qvT_d[:, :])
        qb = sing.tile([P, QV * NQ], BF16)
        nc.sync.dma_start(qb[:], qb_d[:, :])
        wzru = sing.tile([P, QV * P], BF16)
        nc.sync.dma_start(wzru[:], wzru_d[:, :])
        wzc = sing.tile([P, QV * D], BF16)
        nc.sync.dma_start(wzc[:], wzc_d[:, :])
        smalls = sing.tile([QV, 384], BF16)
        nc.sync.dma_start(smalls[:], small_d[:, :])

        # ---------------- qk: per head-pair packed matmul -----------------
        # psum rows per pair: [k_h0(16)@0 .. q_h0(16)@32 .. k_h1@64 q_h1@96]
        kT = [sing.tile([KD, N], BF16, tag=f"kT{h}", name=f"kT{h}")
              for h in range(H)]
        qT = [sing.tile([KD, NQ], BF16, tag=f"qT{h}", name=f"qT{h}")
              for h in range(H)]
        for p_ in range(2):
            ps = psS.tile([P, N], F32, tag="ps", name="qk")
            for half in range(2):
                nc.tensor.matmul(ps[:, ts(half, NQ)], kqw[:, ts(p_, P)],
                                 combT[:, ts(half, NQ)], start=True, stop=False)
                nc.tensor.matmul(ps[:, ts(half, NQ)], kqwL[:, ts(p_, P)],
                                 cxr[:, ts(half, NQ)], start=False, stop=True)
            for hh in range(2):
                h = 2 * p_ + hh
                nc.vector.scalar_tensor_tensor(
                    kT[h][:], ps[64 * hh:64 * hh + KD, :], bkq[:, h:h + 1],
                    zeroN[0:KD, :], op0=ALU.add, op1=ALU.add)
                nc.vector.scalar_tensor_tensor(
                    qT[h][:], ps[64 * hh + 32:64 * hh + 48, 0:NQ],
                    bkq[:, H + h:H + h + 1], zeroN[0:KD, 0:NQ],
                    op0=ALU.add, op1=ALU.add)

        # ---------------- V phase: U = comb @ (Wv W1), all heads ----------
        vt = [sing.tile([P, H, 130], BF16, tag=f"vt{i}", name=f"vt{i}")
              for i in range(NT)]
        for i in range(NT):
            pv = psS.tile([P, N], F32, tag="ps", name="pv")
            for g, off in ((0, 0), (1, NQ)):
                nc.tensor.matmul(pv[:, off:off + 258], combT[:, ts(i, P)],
                                 wv1a[:, g * 258:(g + 1) * 258],
                                 start=True, stop=False)
                nc.tensor.matmul(pv[:, off:off + 258], cxr[0:1, ts(i, P)],
                                 wv1b[:, g * 258:(g + 1) * 258],
                                 start=False, stop=True)
            nc.vector.tensor_copy(
                vt[i][:, :, 0:129].rearrange("p (a h) c -> p a h c", a=2),
                pv[:].rearrange("p (a b) -> p a b", b=NQ)[:, :, 0:258]
                .rearrange("p a (h c) -> p a h c", c=129))
            nc.gpsimd.memset(vt[i][:, :, 129:130], 1.0)

        # ---------------- attention + per-pair softmax norm ---------------
        m1acc = sing.tile([P, NQ], F32, name="m1acc")
        crs = [sing.tile([2, NQ], F32, tag=f"crs{p_}", name=f"crs{p_}")
               for p_ in range(2)]
        unit = 0
        for p_ in range(2):
            Y0 = psY.tile([P, NQ], F32, tag="Y", name="Y0")
            Y1 = psY.tile([P, NQ], F32, tag="Y", name="Y1")
            E0 = psE.tile([2, NQ], F32, tag="E", name="E0")
            E1 = psE.tile([2, NQ], F32, tag="E", name="E1")
            h0, h1 = 2 * p_, 2 * p_ + 1
            for i in range(NT):
                ps = psS.tile([P, N], F32, tag="ps", name="sc")
                nc.tensor.matmul(ps[:, 0:NQ], kT[h0][:, ts(i, P)], qT[h0][:],
                                 start=True, stop=True)
                nc.tensor.matmul(ps[:, NQ:N], kT[h1][:, ts(i, P)], qT[h1][:],
                                 start=True, stop=True)
                sm = smp.tile([P, N], BF16, tag="sm", name="sm")
                if hw_leaky:
                    nc.scalar.activation(sm[:], ps[:], ACTF.Prelu, alpha=0.2)
                else:
                    nc.vector.scalar_tensor_tensor(sm[:], ps[:], 0.2, ps[:],
                                                   op0=ALU.mult, op1=ALU.max)
                unit += 1
                pe = pep.tile([P, N], BF16, tag="pe", name="pe")
                nc.scalar.activation(pe[:], sm[:], ACTF.Exp)
                nc.vector.tensor_tensor(
                    pe[:].rearrange("p (a b) -> p a b", b=NQ),
                    pe[:].rearrange("p (a b) -> p a b", b=NQ),
                    adjT[:, None, ts(i, NQ)].to_broadcast((P, 2, NQ)),
                    ALU.mult)
                st, sp = i == 0, i == NT - 1
                nc.tensor.matmul(Y0[:], vt[i][:, h0, 0:P], pe[:, 0:NQ],
                                 start=st, stop=sp)
                nc.tensor.matmul(E0[:], vt[i][:, h0, P:130], pe[:, 0:NQ],
                                 start=st, stop=sp)
                nc.tensor.matmul(Y1[:], vt[i][:, h1, 0:P], pe[:, NQ:N],
                                 start=st, stop=sp)
                nc.tensor.matmul(E1[:], vt[i][:, h1, P:130], pe[:, NQ:N],
                                 start=st, stop=sp)
            # per-pair normalization (frees Y/E psums for the next pair)
            ex0 = work.tile([2, NQ], F32, tag="ex", name="ex0")
            ex1 = work.tile([2, NQ], F32, tag="ex", name="ex1")
            nc.vector.tensor_copy(ex0[:], E0[:])
            nc.vector.tensor_copy(ex1[:], E1[:])
            dc = sing.tile([2, NQ], F32, tag=f"dc{p_}", name=f"dc{p_}")
            cc = sing.tile([2, NQ], F32, tag=f"cc{p_}", name=f"cc{p_}")
            nc.gpsimd.dma_start(dc[0:1, :], ex0[1:2, :])
            nc.gpsimd.dma_start(dc[1:2, :], ex1[1:2, :])
            nc.gpsimd.dma_start(cc[0:1, :], ex0[0:1, :])
            nc.gpsimd.dma_start(cc[1:2, :], ex1[0:1, :])
            rinv = sing.tile([2, NQ], F32, tag=f"ri{p_}", name=f"ri{p_}")
            nc.vector.reciprocal_approx_fast(rinv[:], dc[:])
            nc.vector.tensor_tensor(crs[p_][:], cc[:], rinv[:], ALU.mult)
            ri1 = sing.tile([1, NQ], F32, tag=f"ri1{p_}", name=f"ri1{p_}")
            nc.gpsimd.dma_start(ri1[:], rinv[1:2, :])
            for hh, Yh in ((0, Y0), (1, Y1)):
                rb = work.tile([P, NQ], F32, tag="rb", name="rb")
                nc.gpsimd.partition_broadcast(
                    rb[:], rinv[0:1, :] if hh == 0 else ri1[:])
                if p_ == 0 and hh == 0:
                    nc.vector.tensor_tensor(m1acc[:], Yh[:], rb[:], ALU.mult)
                else:
                    t_ = work.tile([P, NQ], F32, tag="nt", name="nt")
                    nc.vector.tensor_tensor(t_[:], Yh[:], rb[:], ALU.mult)
                    nc.vector.tensor_tensor(m1acc[:], m1acc[:], t_[:], ALU.add)

        # ---------------- MLP channel 128 + relu + W2 + residual ----------
        c4s = sing.tile([2, NQ], F32, name="c4s")
        nc.vector.tensor_tensor(c4s[:], crs[0][:], crs[1][:], ALU.add)
        nc.gpsimd.partition_all_reduce(c4s[:], c4s[:], 2, bass_isa.ReduceOp.add)
        m1T = sing.tile([P, NQ], BF16, name="m1T")
        nc.scalar.activation(m1T[:], m1acc[:], ACTF.Relu, bias=biases[:, 0:1])
        m1L = sing.tile([1, NQ], BF16, name="m1L")
        nc.scalar.activation(m1L[:], c4s[0:1, :], ACTF.Relu,
                             bias=biasesL[0:1, 0:1])
        pcf = psY.tile([P, NQ], F32, tag="Y", name="pcf")
        nc.tensor.matmul(pcf[:], w2a[:, 0:P], m1T[:], start=True, stop=False)
        nc.tensor.matmul(pcf[:], w2b[:, 0:P], m1L[:], start=False, stop=True)
        pcfL = psE.tile([2, NQ], F32, tag="E", name="pcfL")
        nc.tensor.matmul(pcfL[0:1, :], w2a[:, P:C], m1T[:], start=True, stop=False)
        nc.tensor.matmul(pcfL[0:1, :], w2b[:, P:C], m1L[:], start=False, stop=True)
        cf = sing.tile([P, NQ], BF16, name="cf")
        nc.vector.scalar_tensor_tensor(cf[:], pcf[:], biases[:, 1:2],
                                       combT[:, 0:NQ], op0=ALU.add, op1=ALU.add)
        cl_row = sing.tile([1, NQ], BF16, name="cl_row")
        nc.vector.scalar_tensor_tensor(cl_row[:], pcfL[0:1, :],
                                       biasesL[0:1, 1:2], cxr[0:1, 0:NQ],
                                       op0=ALU.add, op1=ALU.add)

        # preload the sigmoid/tanh table during the MLP window
        scrap = sing.tile([1, 1], F32, name="scrap")
        nc.scalar.activation(scrap[:], biasesL[0:1, 0:1], ACTF.Sigmoid)

        # ---------------- hypernetwork (z-trick, all transposed) ----------
        zq = sing.tile([P, QV * NQ], BF16, name="zq")

        def build_z(selT):
            for g in range(8):
                nc.vector.tensor_tensor(
                    zq[:, g * 2048:(g + 1) * 2048].rearrange(
                        "p (a b) -> p a b", b=NQ),
                    selT[:, None, 0:NQ].to_broadcast((P, 4, NQ)),
                    qb[:, g * 2048:(g + 1) * 2048].rearrange(
                        "p (a b) -> p a b", b=NQ),
                    ALU.mult)

        def hyper_mms(pdst, nr, wz, wL, bL, qcl_):
            for d in range(QV):
                g, dd = d // 4, d % 4
                nc.tensor.matmul(pdst, wz[:, d * nr:(d + 1) * nr],
                                 zq[:, g * 2048 + dd * NQ:
                                    g * 2048 + (dd + 1) * NQ],
                                 start=(d == 0), stop=False)
            nc.tensor.matmul(pdst, wL, qcl_[:], start=False, stop=False)
            nc.tensor.matmul(pdst, bL, qvT[:], start=False, stop=True)

        # r | u
        rep_cl = sing.tile([QV, NQ], BF16, name="rep_cl")
        nc.gpsimd.partition_broadcast(rep_cl[:], cl_row[:])
        qcl = sing.tile([QV, NQ], BF16, name="qcl")
        nc.vector.tensor_tensor(qcl[:], qvT[:], rep_cl[:], ALU.mult)
        build_z(cf)
        pru = psY.tile([P, NQ], F32, tag="Y", name="pru")
        hyper_mms(pru[:], P, wzru, smalls[:, 0:P], smalls[:, P:256], qcl)
        ru = sing.tile([P, NQ], BF16, name="ru")
        nc.scalar.activation(ru[:], pru[:], ACTF.Sigmoid)

        # selc = [hn (rows 0:64) | x (rows 64:128)], channel 128 = cx2 row 0
        selc = sing.tile([P, NQ], BF16, name="selc")
        nc.vector.tensor_tensor(selc[0:D, :], ru[0:D, :], combT[0:D, 0:NQ],
                                ALU.mult)
        nc.vector.tensor_copy(selc[D:P, :], combT[D:P, 0:NQ])
        rep_c2 = sing.tile([QV, NQ], BF16, name="rep_c2")
        nc.gpsimd.partition_broadcast(rep_c2[:], cxr[0:1, 0:NQ])
        qcl2 = sing.tile([QV, NQ], BF16, name="qcl2")
        nc.vector.tensor_tensor(qcl2[:], qvT[:], rep_c2[:], ALU.mult)
        build_z(selc)
        pc = psY.tile([P, NQ], F32, tag="Y", name="pc")
        hyper_mms(pc[0:D, :], D, wzc, smalls[:, 256:320], smalls[:, 320:384],
                  qcl2)
        cand = sing.tile([D, NQ], BF16, name="cand")
        nc.scalar.activation(cand[:], pc[0:D, :], ACTF.Tanh)

        # out = hn + u*(cand - hn)   (hn = selc rows 0:64, u = ru rows 64:128)
        u64 = sing.tile([D, NQ], BF16, name="u64")
        nc.vector.tensor_copy(u64[:], ru[D:P, :])
        d1 = sing.tile([D, NQ], BF16, name="d1")
        nc.vector.tensor_tensor(d1[:], cand[:], selc[0:D, :], ALU.subtract)
        nc.vector.tensor_tensor(d1[:], d1[:], u64[:], ALU.mult)
        outT = sing.tile([D, NQ], F32, name="outT")
        nc.vector.tensor_tensor(outT[:], d1[:], selc[0:D, :], ALU.add)
        for j in range(4):
            pt = psY.tile([P, D], F32, tag="Y", name="pt")
            nc.tensor.transpose(pt[:, 0:D], outT[:, ts(j, P)],
                                identf[0:D, 0:D])
            ob = work.tile([P, D], F32, tag="ob", name="ob")
            nc.vector.tensor_copy(ob[:], pt[:, 0:D])
            nc.sync.dma_start(out_d[ts(j, P), :], ob[:])

    return nc


_NC_CACHE = {}


def _get_nc():
    key = bool(USE_LRELU[0])
    if key not in _NC_CACHE:
        nc = build_graph(hw_leaky=key)
        if not nc.is_finalized():
            nc.finalize()
        _NC_CACHE[key] = nc
    return _NC_CACHE[key]


# channel reorder: new order = [h (64) | x (65)]
_R = np.concatenate([np.arange(65, 129), np.arange(0, 65)])
_BF = ml_dtypes.bfloat16


def _bf(a):
    return np.ascontiguousarray(np.asarray(a, np.float32).astype(_BF))


def _prep_shared(Wq, bq, Wk, bk, Wv, bv, W1, b1, W2, b2, Wr, br, Wu, bu, Wc, bc):
    f32 = np.float32
    Wq, bq = np.asarray(Wq, f32), np.asarray(bq, f32)
    Wk, bk = np.asarray(Wk, f32), np.asarray(bk, f32)
    Wv, bv = np.asarray(Wv, f32), np.asarray(bv, f32)
    W1, b1 = np.asarray(W1, f32).reshape(H, C, C), np.asarray(b1, f32)
    W2, b2 = np.asarray(W2, f32), np.asarray(b2, f32)

    # qk packed: per pair cols [k_h0|0|q_h0/4|0|k_h1|0|q_h1/4|0] (16 each);
    # contraction rows = 128 reordered channels + c128; biases ride the
    # psum->sbuf copies as per-partition bias APs (bkq).
    Wq_r, Wk_r = Wq[:, _R, :], Wk[:, _R, :]
    kqw = np.zeros((C, 256), f32)
    bkq = np.zeros((KD, 2 * H), f32)
    for h in range(H):
        base = (h // 2) * 128 + (h % 2) * 64
        kqw[0:129, base:base + 16] = Wk_r[h]
        kqw[0:129, base + 32:base + 48] = Wq_r[h] * 0.25
        bkq[:, h] = bk[h]
        bkq[:, H + h] = bq[h] * 0.25

    # V with W1 folded: U_h = Wv_h @ W1_h, contraction rows reordered
    U = np.stack([(Wv[h] @ W1[h])[_R] for h in range(H)])    # [H, 129, 129]
    wv1 = np.ascontiguousarray(
        np.transpose(U, (1, 0, 2)).reshape(C, H * C))        # [129, 516]
    b1_eff = b1 + sum(bv[h] @ W1[h] for h in range(H))

    w2r = W2[:, _R]                                          # cols reordered
    b2r = b2[_R]
    biases = np.zeros((C, 4), f32)
    biases[:, 0] = b1_eff
    biases[:, 1] = b2r

    Wr_r = np.asarray(Wr, f32)[:, _R, :]
    Wu_r = np.asarray(Wu, f32)[:, _R, :]
    Wc_r = np.asarray(Wc, f32)[:, _R, :]
    wzru = np.ascontiguousarray(np.transpose(
        np.concatenate([Wr_r[:, 0:128, :], Wu_r[:, 0:128, :]], 2),
        (1, 0, 2)).reshape(P, QV * P))
    wzc = np.ascontiguousarray(
        np.transpose(Wc_r[:, 0:128, :], (1, 0, 2)).reshape(P, QV * D))
    smalls = np.zeros((QV, 384), f32)
    smalls[:, 0:64] = Wr_r[:, 128, :]
    smalls[:, 64:128] = Wu_r[:, 128, :]
    smalls[:, 128:192] = np.asarray(br, f32)
    smalls[:, 192:256] = np.asarray(bu, f32)
    smalls[:, 256:320] = Wc_r[:, 128, :]
    smalls[:, 320:384] = np.asarray(bc, f32)

    return dict(
        kqw=_bf(kqw[0:128]), kqwL=_bf(kqw[128:129]),
        bkq=np.ascontiguousarray(bkq),
        wv1a=_bf(wv1[0:128]), wv1b=_bf(wv1[128:129]),
        w2a=_bf(w2r[0:128]), w2b=_bf(w2r[128:129]),
        biases=np.ascontiguousarray(biases[0:128]),
        biasesL=np.ascontiguousarray(biases[128:129]),
        wzru=_bf(wzru), wzc=_bf(wzc), smalls=_bf(smalls),
    )


def _prep_core(b, x, h, query_vectors, adj, nodes_n, shared):
    idx = nodes_n[b * NQ:(b + 1) * NQ].astype(np.int64)
    rest = np.setdiff1d(np.arange(N, dtype=np.int64), idx)
    perm = np.concatenate([idx, rest])
    comb = np.concatenate([x[b][perm], h[b][perm]], 1)[:, _R]  # [N,129] reord
    combT = np.ascontiguousarray(comb.T.astype(_BF))           # [129, N]
    qv = query_vectors[b * NQ:(b + 1) * NQ]                    # [512, 32]
    qvT = np.ascontiguousarray(qv.T.astype(_BF))               # [32, 512]
    # qb [128, 32*512]: qb[p, d*512 + m] = qv[m, d]  (uint16 view = fast)
    qvT_u16 = qvT.view(np.uint16)
    qb = np.ascontiguousarray(
        np.broadcast_to(qvT_u16[None, :, :], (P, QV, NQ))
        .reshape(P, QV * NQ)).view(_BF)
    adj01 = (adj[np.ix_(idx, perm)] != 0).astype(np.float32).T  # [N, 512]
    adjT = np.ascontiguousarray(
        adj01.reshape(NT, P, NQ).transpose(1, 0, 2).reshape(P, NT * NQ)
    ).astype(_BF)
    d = dict(shared)
    d["combT"] = np.ascontiguousarray(combT[0:128])
    d["c128"] = np.ascontiguousarray(combT[128:129])
    d["qvT"] = qvT
    d["qb"] = qb
    d["adjT"] = adjT
    return d


def make_in_maps(x, h, query_vectors, adj, nodes_b, nodes_n, **weights):
    x = np.asarray(x, np.float32)
    h = np.asarray(h, np.float32)
    query_vectors = np.asarray(query_vectors, np.float32)
    adj = np.asarray(adj)
    nodes_n = np.asarray(nodes_n)
    shared = _prep_shared(**weights)
    return [_prep_core(b, x, h, query_vectors, adj, nodes_n, shared)
            for b in range(B)]


def kernel(x, h, query_vectors, adj, nodes_b, nodes_n,
           Wq, bq, Wk, bk, Wv, bv, W1, b1, W2, b2,
           Wr, br, Wu, bu, Wc, bc):
    in_maps = make_in_maps(
        x, h, query_vectors, adj, nodes_b, nodes_n,
        Wq=Wq, bq=bq, Wk=Wk, bk=bk, Wv=Wv, bv=bv, W1=W1, b1=b1, W2=W2, b2=b2,
        Wr=Wr, br=br, Wu=Wu, bu=bu, Wc=Wc, bc=bc)
    nc = _get_nc()
    res = run_bass_kernel_spmd(nc, in_maps, list(range(B)))
    outs = [np.asarray(res.results[b]["out"], np.float32) for b in range(B)]
    return np.concatenate(outs, axis=0)
